# revision 1
# baseline (speedup 1.0000x reference)
import sys

import numpy as np

sys.path.insert(0, "/opt/trn_rl_repo")

from concourse import bacc, bass, tile  # noqa: E402,F401
from concourse import mybir  # noqa: E402
from concourse.bass import broadcast_tensor_aps  # noqa: E402
from concourse.bass_utils import run_bass_kernel_spmd  # noqa: E402

N_CORES = 8
S = 8  # samples per core
C = 3
T = 9
H = W = 256
RC = 4  # rows per chunk (one SBUF partition holds one chunk)
NCH = H // RC  # 64 chunks per sample
RP = RC + 2  # row slots incl top/bottom halo
WP = W + 2  # col slots incl left/right reflect pad
F32 = mybir.dt.float32
F16 = mybir.dt.float16
NPROD = 4  # product ring depth
# center tap first (needs no halo rows / col pads), then row-halo-only
# taps, then col-pad taps, corners last: first mul waits only on the
# 6 channel DMAs + one sigma tap instead of all x DMAs + pads
TAP_ORDER = [4, 1, 7, 3, 5, 0, 2, 6, 8]


def build_nc():
    nc = bacc.Bacc()
    x_ext = nc.declare_dram_parameter("x", [S, C, H, W], F16, isOutput=False)
    sg_ext = nc.declare_dram_parameter("sigma", [S, T, H, W], F16, isOutput=False)
    out_ext = nc.declare_dram_parameter("out", [S, C, H, W], F32, isOutput=True)

    with tile.TileContext(nc) as tc:
        with tc.tile_pool(name="p", bufs=2) as pool:
            for stripe in range(S // 2):
                xt = pool.tile([128, C, RP, WP], F16)
                st = pool.tile([128, T, RC, W], F16)
                prods = [
                    pool.tile([128, C, RC, W], F16, name=f"prod{j}")
                    for j in range(NPROD)
                ]
                acc = pool.tile([128, C, RC, W], F16)
                den16 = pool.tile([128, RC, W], F16)
                den = pool.tile([128, 1, RC, W], F32)
                inv = pool.tile([128, 1, RC, W], F32)
                ot = pool.tile([128, C, RC, W], F32)

                for k in range(2):
                    s = 2 * stripe + k
                    pb = 64 * k
                    # disjoint partition halves -> run the two samples' DMAs
                    # on separate engine queues
                    eng = nc.sync if k == 0 else nc.scalar
                    xr = x_ext[s].rearrange("c (n r) w -> n c r w", r=RC)
                    sr = sg_ext[s].rearrange("t (n r) w -> n t r w", r=RC)
                    # main rows -> slots 1..RC, image cols -> slots 1..W
                    # (DMA APs are limited to 3 dims -> one DMA per channel)
                    for c in range(C):
                        eng.dma_start(
                            xt[pb : pb + 64, c, 1 : 1 + RC, 1 : 1 + W], xr[:, c]
                        )
                    # center tap's sigma right after the mains: its mul
                    # needs neither halos nor pads
                    eng.dma_start(
                        st[pb : pb + 64, TAP_ORDER[0]], sr[:, TAP_ORDER[0]]
                    )
                    # top halo row: chunks 1..63 read prev chunk row 3
                    eng.dma_start(
                        xt[pb + 1 : pb + 64, :, 0, 1 : 1 + W], xr[0:63, :, 3, :]
                    )
                    # chunk 0 top halo: reflect row 1
                    eng.dma_start(xt[pb : pb + 1, :, 0, 1 : 1 + W], xr[0:1, :, 1, :])
                    # bottom halo row: chunks 0..62 read next chunk row 0
                    eng.dma_start(xt[pb : pb + 63, :, 5, 1 : 1 + W], xr[1:64, :, 0, :])
                    # chunk 63 bottom halo: reflect row 254 (= chunk 63 row 2)
                    eng.dma_start(
                        xt[pb + 63 : pb + 64, :, 5, 1 : 1 + W], xr[63:64, :, 2, :]
                    )
                    # remaining sigma taps streamed in consumption order
                    for t in TAP_ORDER[1:]:
                        eng.dma_start(st[pb : pb + 64, t], sr[:, t])

                # column reflect pads: slot 0 <- image col 1 (slot 2),
                # slot WP-1 <- image col W-2 (slot WP-3)
                nc.scalar.copy(xt[:, :, :, 0:1], xt[:, :, :, 2:3])
                nc.scalar.copy(xt[:, :, :, WP - 1 : WP], xt[:, :, :, WP - 3 : WP - 2])

                # All compute on DVE: gpsimd touching recycled pool buffers
                # faults HW (NRT_EXEC_UNIT_UNRECOVERABLE 101). fp16 keeps
                # DVE in 2x_1p perf mode.
                with nc.allow_low_precision(reason="fp16 kernel"):
                    for j, t in enumerate(TAP_ORDER):
                        di, dj = t // 3, t % 3
                        xs = xt[:, :, di : di + RC, dj : dj + W]
                        sg = st[:, t : t + 1]
                        a, b = broadcast_tensor_aps(xs, sg)
                        nc.vector.tensor_mul(prods[j % NPROD][:], a, b)
                        if j == 1:
                            nc.vector.tensor_add(acc[:], prods[0][:], prods[1][:])
                        elif j > 1:
                            nc.vector.tensor_add(
                                acc[:], acc[:], prods[j % NPROD][:]
                            )

                    nc.vector.tensor_add(den16[:], st[:, 0], st[:, 1])
                    for t in range(2, T - 1):
                        nc.vector.tensor_add(den16[:], den16[:], st[:, t])
                nc.vector.tensor_add(den[:, 0], den16[:], st[:, T - 1])
                # ~5x faster than reciprocal(); ~18 correct bits >> fp16
                # noise floor, den in [0.8, 9] so no edge cases
                nc.vector.reciprocal_approx_fast(inv[:, 0], den[:, 0])

                # normalize + store per channel: out DMA of channel c starts
                # while channel c+1 is still normalizing (shrinks the tail)
                for c in range(C):
                    nc.vector.tensor_mul(ot[:, c], acc[:, c], inv[:, 0])
                    for k in range(2):
                        s = 2 * stripe + k
                        pb = 64 * k
                        eng = nc.sync if k == 0 else nc.scalar
                        orr = out_ext[s].rearrange("c (n r) w -> n c r w", r=RC)
                        eng.dma_start(orr[:, c], ot[pb : pb + 64, c])

    nc.finalize()
    return nc


_nc_cache = None


def _get_nc():
    global _nc_cache
    if _nc_cache is None:
        _nc_cache = build_nc()
    return _nc_cache


def _run(x, sigma, trace=False):
    x = np.ascontiguousarray(x).astype(np.float16)
    sigma = np.ascontiguousarray(sigma).astype(np.float16)
    nc = _get_nc()
    in_maps = [
        {"x": x[S * i : S * (i + 1)], "sigma": sigma[S * i : S * (i + 1)]}
        for i in range(N_CORES)
    ]
    res = run_bass_kernel_spmd(nc, in_maps, list(range(N_CORES)), trace=trace)
    out = np.concatenate([res.results[i]["out"] for i in range(N_CORES)], axis=0)
    return out.astype(np.float32, copy=False), res


def kernel(x, sigma):
    out, _ = _run(x, sigma)
    return out



# revision 2
# speedup vs baseline: 1.0222x; 1.0222x over previous
import sys

import numpy as np

sys.path.insert(0, "/opt/trn_rl_repo")

from concourse import bacc, bass, tile  # noqa: E402,F401
from concourse import mybir  # noqa: E402
from concourse.bass import broadcast_tensor_aps  # noqa: E402
from concourse.bass_utils import run_bass_kernel_spmd  # noqa: E402
from concourse.masks import make_identity  # noqa: E402

N_CORES = 8
S = 8  # samples per core
C = 3
T = 9
H = W = 256
RC = 8  # rows per chunk (one SBUF partition holds one chunk)
NCH = H // RC  # 32 chunks per sample
SPS = 128 // NCH  # 4 samples striped across the 128 partitions
NSTRIPES = S // SPS  # 2
RP = RC + 2  # row slots incl top/bottom halo
WP = W + 2  # col slots incl left/right reflect pad
HR = 4  # rows per PSUM block (half a chunk): 3ch acc + den = 8 banks
HPX = HR * W  # 1024 psum columns per block
F32 = mybir.dt.float32
F16 = mybir.dt.float16
# center tap first (needs no halo rows / col pads), then row-halo-only
# taps, then col-pad taps, corners last: first mul waits only on the
# main-row DMAs + one sigma tap instead of all x DMAs + pads
TAP_ORDER = [4, 1, 7, 3, 5, 0, 2, 6, 8]


def build_nc():
    nc = bacc.Bacc()
    x_ext = nc.declare_dram_parameter("x", [S, C, H, W], F16, isOutput=False)
    sg_ext = nc.declare_dram_parameter("sigma", [S, T, H, W], F16, isOutput=False)
    out_ext = nc.declare_dram_parameter("out", [S, C, H, W], F16, isOutput=True)

    with tile.TileContext(nc) as tc:
        with (
            tc.tile_pool(name="const", bufs=1) as cpool,
            tc.psum_pool(name="ps", bufs=1) as pspool,
            tc.tile_pool(name="io", bufs=2) as iopool,
            tc.tile_pool(name="pr", bufs=3) as prpool,
            tc.tile_pool(name="sc", bufs=2) as scpool,
        ):
            # stationary identity: matmul(I, prod) == copy-with-accumulate
            # into PSUM, so the PE does all tap + denominator summation
            ident = cpool.tile([128, 128], F16)
            make_identity(nc, ident)

            acc = pspool.tile([128, C, HPX], F32)  # 6 psum banks
            den = pspool.tile([128, HPX], F32)  # 2 psum banks

            for stripe in range(NSTRIPES):
                xt = iopool.tile([128, C, RP, WP], F16)
                st = iopool.tile([128, T, RC, W], F16)
                ot = iopool.tile([128, C, RC, W], F16)

                for k in range(SPS):
                    s = SPS * stripe + k
                    pb = NCH * k
                    # split the samples' DMAs across both HWDGE queues
                    eng = nc.sync if k % 2 == 0 else nc.scalar
                    xr = x_ext[s].rearrange("c (n r) w -> n c r w", r=RC)
                    sr = sg_ext[s].rearrange("t (n r) w -> n t r w", r=RC)
                    # center tap's sigma first: the first muls need it
                    eng.dma_start(
                        st[pb : pb + NCH, TAP_ORDER[0]], sr[:, TAP_ORDER[0]]
                    )
                    # main rows -> slots 1..RC, image cols -> slots 1..W
                    for c in range(C):
                        eng.dma_start(
                            xt[pb : pb + NCH, c, 1 : 1 + RC, 1 : 1 + W], xr[:, c]
                        )
                    # top halo row: chunks 1.. read prev chunk's last row
                    eng.dma_start(
                        xt[pb + 1 : pb + NCH, :, 0, 1 : 1 + W],
                        xr[0 : NCH - 1, :, RC - 1, :],
                    )
                    # chunk 0 top halo: reflect row 1
                    eng.dma_start(
                        xt[pb : pb + 1, :, 0, 1 : 1 + W], xr[0:1, :, 1, :]
                    )
                    # bottom halo row: chunks ..NCH-2 read next chunk's row 0
                    eng.dma_start(
                        xt[pb : pb + NCH - 1, :, RP - 1, 1 : 1 + W],
                        xr[1:NCH, :, 0, :],
                    )
                    # last chunk bottom halo: reflect row H-2
                    eng.dma_start(
                        xt[pb + NCH - 1 : pb + NCH, :, RP - 1, 1 : 1 + W],
                        xr[NCH - 1 : NCH, :, RC - 2, :],
                    )
                    # remaining sigma taps streamed in consumption order
                    for t in TAP_ORDER[1:]:
                        eng.dma_start(st[pb : pb + NCH, t], sr[:, t])

                # column reflect pads: slot 0 <- image col 1 (slot 2),
                # slot WP-1 <- image col W-2 (slot WP-3)
                nc.scalar.copy(xt[:, :, :, 0:1], xt[:, :, :, 2:3])
                nc.scalar.copy(xt[:, :, :, WP - 1 : WP], xt[:, :, :, WP - 3 : WP - 2])

                for half in range(RC // HR):
                    r0 = HR * half
                    with nc.allow_low_precision(reason="fp16 products"):
                        for j, t in enumerate(TAP_ORDER):
                            di, dj = t // 3, t % 3
                            prod = prpool.tile([128, C, HR, W], F16, name="prod")
                            xs = xt[:, :, r0 + di : r0 + di + HR, dj : dj + W]
                            sg = st[:, t : t + 1, r0 : r0 + HR, :]
                            a, b = broadcast_tensor_aps(xs, sg)
                            nc.vector.tensor_mul(prod[:], a, b)
                            first, last = j == 0, j == T - 1
                            for c in range(C):
                                for bk in range(HPX // 512):
                                    nc.tensor.matmul(
                                        acc[:, c, 512 * bk : 512 * (bk + 1)],
                                        ident[:],
                                        prod[:, c, 2 * bk : 2 * (bk + 1), :],
                                        start=first,
                                        stop=last,
                                    )
                            for bk in range(HPX // 512):
                                nc.tensor.matmul(
                                    den[:, 512 * bk : 512 * (bk + 1)],
                                    ident[:],
                                    st[:, t, r0 + 2 * bk : r0 + 2 * (bk + 1), :],
                                    start=first,
                                    stop=last,
                                )
                    inv = scpool.tile([128, 1, HPX], F32)
                    # ~18 correct bits >> fp16 noise floor; den in (0, 9]
                    nc.vector.reciprocal_approx_fast(inv[:, 0], den[:])
                    otv = ot[:, :, r0 : r0 + HR, :].rearrange("p c r w -> p c (r w)")
                    a, b = broadcast_tensor_aps(acc[:], inv[:])
                    with nc.allow_low_precision(reason="fp16 output"):
                        nc.vector.tensor_mul(otv, a, b)

                for k in range(SPS):
                    s = SPS * stripe + k
                    pb = NCH * k
                    eng = nc.sync if k % 2 == 0 else nc.scalar
                    orr = out_ext[s].rearrange("c (n r) w -> n c r w", r=RC)
                    eng.dma_start(orr, ot[pb : pb + NCH])

    nc.finalize()
    return nc


_nc_cache = None


def _get_nc():
    global _nc_cache
    if _nc_cache is None:
        _nc_cache = build_nc()
    return _nc_cache


def _run(x, sigma, trace=False):
    x = np.ascontiguousarray(x).astype(np.float16)
    sigma = np.ascontiguousarray(sigma).astype(np.float16)
    nc = _get_nc()
    in_maps = [
        {"x": x[S * i : S * (i + 1)], "sigma": sigma[S * i : S * (i + 1)]}
        for i in range(N_CORES)
    ]
    res = run_bass_kernel_spmd(nc, in_maps, list(range(N_CORES)), trace=trace)
    out = np.concatenate([res.results[i]["out"] for i in range(N_CORES)], axis=0)
    return out.astype(np.float32, copy=False), res


def kernel(x, sigma):
    out, _ = _run(x, sigma)
    return out


# revision 3
# speedup vs baseline: 1.0850x; 1.0614x over previous
import sys

import numpy as np

sys.path.insert(0, "/opt/trn_rl_repo")

from concourse import bacc, bass, tile  # noqa: E402,F401
from concourse import mybir  # noqa: E402
from concourse.bass import broadcast_tensor_aps  # noqa: E402
from concourse.bass_utils import run_bass_kernel_spmd  # noqa: E402
from concourse.masks import make_identity  # noqa: E402

N_CORES = 8
S = 8  # samples per core
C = 3
T = 9
H = W = 256
RC = 8  # rows per chunk (one SBUF partition holds one chunk)
NCH = H // RC  # 32 chunks per sample
SPS = 128 // NCH  # 4 samples striped across the 128 partitions
NSTRIPES = S // SPS  # 2
RP = RC + 2  # row slots incl top/bottom halo
WP = W + 2  # col slots incl left/right reflect pad
HR = 4  # rows per mul tile (half a chunk)
BR = 2  # rows per PSUM block: 3ch acc + den = 4 banks, 2 sets ping-pong
BPX = BR * W  # 512 psum columns per block
F32 = mybir.dt.float32
F16 = mybir.dt.float16
# Per-half tap order: center tap first (no halo rows / col pads), then
# the two di=1 col-pad taps, then the di needing no halo for this half,
# finally the di that reads this half's halo row.  Matches the merged
# sigma DMA groups {4}, {3,5}, {6,7,8}, {0,1,2} in issue order.
TAPS_H0 = [4, 5, 3, 8, 7, 6, 2, 1, 0]
TAPS_H1 = [4, 5, 3, 2, 1, 0, 8, 7, 6]


def build_nc():
    nc = bacc.Bacc()
    x_ext = nc.declare_dram_parameter("x", [S, C, H, W], F16, isOutput=False)
    sg_ext = nc.declare_dram_parameter("sigma", [S, T, H, W], F16, isOutput=False)
    out_ext = nc.declare_dram_parameter("out", [S, C, H, W], F16, isOutput=True)

    with tile.TileContext(nc) as tc:
        with (
            tc.tile_pool(name="const", bufs=1) as cpool,
            tc.psum_pool(name="ps", bufs=1) as pspool,
            tc.tile_pool(name="io", bufs=2) as iopool,
            tc.tile_pool(name="pr", bufs=3) as prpool,
            tc.tile_pool(name="sc", bufs=2) as scpool,
        ):
            # stationary identity: matmul(I, prod) == copy-with-accumulate
            # into PSUM, so the PE does all tap + denominator summation
            ident = cpool.tile([128, 128], F16)
            make_identity(nc, ident)

            # two 4-bank sets ping-pong so the PE never waits on evacuation
            accs = [pspool.tile([128, C, BPX], F32, name=f"acc{i}") for i in range(2)]
            dens = [pspool.tile([128, BPX], F32, name=f"den{i}") for i in range(2)]

            for stripe in range(NSTRIPES):
                xt = iopool.tile([128, C, RP, WP], F16)
                st = iopool.tile([128, T, RC, W], F16)
                ot = iopool.tile([128, C, RC, W], F16)

                def eng(k):
                    return nc.sync if k % 2 == 0 else nc.scalar

                xrs, srs = [], []
                for k in range(SPS):
                    s = SPS * stripe + k
                    xrs.append(x_ext[s].rearrange("c (n r) w -> n c r w", r=RC))
                    srs.append(sg_ext[s].rearrange("t (n r) w -> n t r w", r=RC))

                # issue order tuned so the first muls' inputs land first:
                # sigma center tap, then x main rows, then the rest in
                # consumption order
                for k in range(SPS):
                    pb = NCH * k
                    eng(k).dma_start(st[pb : pb + NCH, 4], srs[k][:, 4])
                for k in range(SPS):
                    pb = NCH * k
                    for c in range(C):
                        eng(k).dma_start(
                            xt[pb : pb + NCH, c, 1 : 1 + RC, 1 : 1 + W], xrs[k][:, c]
                        )
                for k in range(SPS):
                    pb = NCH * k
                    eng(k).dma_start(st[pb : pb + NCH, 3:6:2], srs[k][:, 3:6:2])
                # col reflect pads for the main rows (slot 0 <- image col 1,
                # slot WP-1 <- image col W-2)
                nc.scalar.copy(xt[:, :, 1 : 1 + RC, 0:1], xt[:, :, 1 : 1 + RC, 2:3])
                nc.scalar.copy(
                    xt[:, :, 1 : 1 + RC, WP - 1 : WP], xt[:, :, 1 : 1 + RC, WP - 3 : WP - 2]
                )
                for k in range(SPS):
                    pb = NCH * k
                    xr = xrs[k]
                    # top halo row: chunks 1.. read prev chunk's last row
                    eng(k).dma_start(
                        xt[pb + 1 : pb + NCH, :, 0, 1 : 1 + W],
                        xr[0 : NCH - 1, :, RC - 1, :],
                    )
                    # chunk 0 top halo: reflect row 1
                    eng(k).dma_start(xt[pb : pb + 1, :, 0, 1 : 1 + W], xr[0:1, :, 1, :])
                    # bottom halo row: chunks ..NCH-2 read next chunk's row 0
                    eng(k).dma_start(
                        xt[pb : pb + NCH - 1, :, RP - 1, 1 : 1 + W],
                        xr[1:NCH, :, 0, :],
                    )
                    # last chunk bottom halo: reflect row H-2
                    eng(k).dma_start(
                        xt[pb + NCH - 1 : pb + NCH, :, RP - 1, 1 : 1 + W],
                        xr[NCH - 1 : NCH, :, RC - 2, :],
                    )
                # halo-row col pads
                nc.scalar.copy(xt[:, :, 0:1, 0:1], xt[:, :, 0:1, 2:3])
                nc.scalar.copy(
                    xt[:, :, 0:1, WP - 1 : WP], xt[:, :, 0:1, WP - 3 : WP - 2]
                )
                nc.scalar.copy(xt[:, :, RP - 1 : RP, 0:1], xt[:, :, RP - 1 : RP, 2:3])
                nc.scalar.copy(
                    xt[:, :, RP - 1 : RP, WP - 1 : WP],
                    xt[:, :, RP - 1 : RP, WP - 3 : WP - 2],
                )
                for k in range(SPS):
                    pb = NCH * k
                    eng(k).dma_start(st[pb : pb + NCH, 6:9], srs[k][:, 6:9])
                for k in range(SPS):
                    pb = NCH * k
                    eng(k).dma_start(st[pb : pb + NCH, 0:3], srs[k][:, 0:3])

                for half in range(RC // HR):
                    r0 = HR * half
                    taps = TAPS_H0 if half == 0 else TAPS_H1
                    with nc.allow_low_precision(reason="fp16 products"):
                        for j, t in enumerate(taps):
                            di, dj = t // 3, t % 3
                            prod = prpool.tile([128, C, HR, W], F16, name="prod")
                            xs = xt[:, :, r0 + di : r0 + di + HR, dj : dj + W]
                            sg = st[:, t : t + 1, r0 : r0 + HR, :]
                            a, b = broadcast_tensor_aps(xs, sg)
                            nc.vector.tensor_mul(prod[:], a, b)
                            first, last = j == 0, j == T - 1
                            # den first: frees the reciprocal to start the
                            # moment the last tap's sigma matmul retires
                            for blk in range(2):
                                nc.tensor.matmul(
                                    dens[blk][:],
                                    ident[:],
                                    st[:, t, r0 + BR * blk : r0 + BR * (blk + 1), :],
                                    start=first,
                                    stop=last,
                                )
                            for blk in range(2):
                                for c in range(C):
                                    nc.tensor.matmul(
                                        accs[blk][:, c],
                                        ident[:],
                                        prod[:, c, BR * blk : BR * (blk + 1), :],
                                        start=first,
                                        stop=last,
                                    )
                    for blk in range(2):
                        rb = r0 + BR * blk
                        inv = scpool.tile([128, 1, BPX], F32, name="inv")
                        # ~18 correct bits >> fp16 noise floor; den in (0, 9]
                        nc.vector.reciprocal_approx_fast(inv[:, 0], dens[blk][:])
                        otv = ot[:, :, rb : rb + BR, :].rearrange(
                            "p c r w -> p c (r w)"
                        )
                        a, b = broadcast_tensor_aps(accs[blk][:], inv[:])
                        with nc.allow_low_precision(reason="fp16 output"):
                            nc.vector.tensor_mul(otv, a, b)

                for k in range(SPS):
                    s = SPS * stripe + k
                    pb = NCH * k
                    orr = out_ext[s].rearrange("c (n r) w -> n c r w", r=RC)
                    eng(k).dma_start(orr, ot[pb : pb + NCH])

    nc.finalize()
    return nc


_nc_cache = None


def _get_nc():
    global _nc_cache
    if _nc_cache is None:
        _nc_cache = build_nc()
    return _nc_cache


def _run(x, sigma, trace=False):
    x = np.ascontiguousarray(x).astype(np.float16)
    sigma = np.ascontiguousarray(sigma).astype(np.float16)
    nc = _get_nc()
    in_maps = [
        {"x": x[S * i : S * (i + 1)], "sigma": sigma[S * i : S * (i + 1)]}
        for i in range(N_CORES)
    ]
    res = run_bass_kernel_spmd(nc, in_maps, list(range(N_CORES)), trace=trace)
    out = np.concatenate([res.results[i]["out"] for i in range(N_CORES)], axis=0)
    return out.astype(np.float32, copy=False), res


def kernel(x, sigma):
    out, _ = _run(x, sigma)
    return out


# revision 4
# speedup vs baseline: 1.4621x; 1.3476x over previous
import sys

import numpy as np

sys.path.insert(0, "/opt/trn_rl_repo")

from concourse import bacc, bass, tile  # noqa: E402,F401
from concourse import mybir  # noqa: E402
from concourse.bass import broadcast_tensor_aps  # noqa: E402
from concourse.bass_utils import run_bass_kernel_spmd  # noqa: E402
from concourse.masks import make_identity  # noqa: E402

N_CORES = 8
S = 8  # samples per core
C = 3
T = 9
H = W = 256
RC = 8  # rows per chunk (one SBUF partition holds one chunk)
NCH = H // RC  # 32 chunks per sample
SPS = 128 // NCH  # 4 samples striped across the 128 partitions
NSTRIPES = S // SPS  # 2
RP = RC + 2  # rows per chunk incl halo (materialized on host)
WP = W + 2  # cols incl reflect pad (materialized on host)
HR = 4  # rows per mul tile (half a chunk)
BR = 2  # rows per PSUM block: 3ch acc + den = 4 banks, 2 sets ping-pong
BPX = BR * W  # 512 psum columns per block
F32 = mybir.dt.float32
F16 = mybir.dt.float16
# Per-half tap order: center tap first, then taps in merged-sigma-DMA
# arrival order {4}, {3,5}, {6,7,8}, {0,1,2}
TAPS_H0 = [4, 5, 3, 8, 7, 6, 2, 1, 0]
TAPS_H1 = [4, 5, 3, 2, 1, 0, 8, 7, 6]


def build_nc():
    nc = bacc.Bacc()
    # x arrives pre-padded and chunked on host: per chunk 10 rows x 258
    # cols (reflect halo+pads materialized) so one contiguous DMA per
    # sample loads mains + halos + pads at once
    x_ext = nc.declare_dram_parameter("x", [S, NCH, C, RP, WP], F16, isOutput=False)
    sg_ext = nc.declare_dram_parameter("sigma", [S, T, H, W], F16, isOutput=False)
    out_ext = nc.declare_dram_parameter("out", [S, C, H, W], F16, isOutput=True)

    with tile.TileContext(nc) as tc:
        with (
            tc.tile_pool(name="const", bufs=1) as cpool,
            tc.psum_pool(name="ps", bufs=1) as pspool,
            tc.tile_pool(name="io", bufs=2) as iopool,
            tc.tile_pool(name="pr", bufs=3) as prpool,
            tc.tile_pool(name="sc", bufs=2) as scpool,
        ):
            # stationary identity: matmul(I, prod) == copy-with-accumulate
            # into PSUM, so the PE does all tap + denominator summation
            ident = cpool.tile([128, 128], F16)
            make_identity(nc, ident)

            # two 4-bank sets ping-pong so the PE never waits on evacuation
            accs = [pspool.tile([128, C, BPX], F32, name=f"acc{i}") for i in range(2)]
            dens = [pspool.tile([128, BPX], F32, name=f"den{i}") for i in range(2)]

            for stripe in range(NSTRIPES):
                xt = iopool.tile([128, C, RP, WP], F16)
                st = iopool.tile([128, T, RC, W], F16)
                ot = iopool.tile([128, C, RC, W], F16)

                def eng(k):
                    return nc.sync if k % 2 == 0 else nc.scalar

                srs = []
                for k in range(SPS):
                    s = SPS * stripe + k
                    srs.append(sg_ext[s].rearrange("t (n r) w -> n t r w", r=RC))

                # issue order tuned so the first muls' inputs land first
                for k in range(SPS):
                    pb = NCH * k
                    eng(k).dma_start(st[pb : pb + NCH, 4], srs[k][:, 4])
                for k in range(SPS):
                    pb = NCH * k
                    s = SPS * stripe + k
                    eng(k).dma_start(xt[pb : pb + NCH], x_ext[s])
                for k in range(SPS):
                    pb = NCH * k
                    eng(k).dma_start(st[pb : pb + NCH, 3:6:2], srs[k][:, 3:6:2])
                for k in range(SPS):
                    pb = NCH * k
                    eng(k).dma_start(st[pb : pb + NCH, 6:9], srs[k][:, 6:9])
                for k in range(SPS):
                    pb = NCH * k
                    eng(k).dma_start(st[pb : pb + NCH, 0:3], srs[k][:, 0:3])

                for half in range(RC // HR):
                    r0 = HR * half
                    taps = TAPS_H0 if half == 0 else TAPS_H1
                    with nc.allow_low_precision(reason="fp16 products"):
                        for j, t in enumerate(taps):
                            di, dj = t // 3, t % 3
                            prod = prpool.tile([128, C, HR, W], F16, name="prod")
                            xs = xt[:, :, r0 + di : r0 + di + HR, dj : dj + W]
                            sg = st[:, t : t + 1, r0 : r0 + HR, :]
                            a, b = broadcast_tensor_aps(xs, sg)
                            nc.vector.tensor_mul(prod[:], a, b)
                            first, last = j == 0, j == T - 1
                            # den first: frees the reciprocal to start the
                            # moment the last tap's sigma matmul retires
                            for blk in range(2):
                                nc.tensor.matmul(
                                    dens[blk][:],
                                    ident[:],
                                    st[:, t, r0 + BR * blk : r0 + BR * (blk + 1), :],
                                    start=first,
                                    stop=last,
                                )
                            for blk in range(2):
                                for c in range(C):
                                    nc.tensor.matmul(
                                        accs[blk][:, c],
                                        ident[:],
                                        prod[:, c, BR * blk : BR * (blk + 1), :],
                                        start=first,
                                        stop=last,
                                    )
                    for blk in range(2):
                        rb = r0 + BR * blk
                        inv = scpool.tile([128, 1, BPX], F32, name="inv")
                        # ~18 correct bits >> fp16 noise floor; den in (0, 9]
                        nc.vector.reciprocal_approx_fast(inv[:, 0], dens[blk][:])
                        otv = ot[:, :, rb : rb + BR, :].rearrange(
                            "p c r w -> p c (r w)"
                        )
                        a, b = broadcast_tensor_aps(accs[blk][:], inv[:])
                        with nc.allow_low_precision(reason="fp16 output"):
                            nc.vector.tensor_mul(otv, a, b)
                    # store this half while the next one computes
                    for k in range(SPS):
                        s = SPS * stripe + k
                        pb = NCH * k
                        orr = out_ext[s].rearrange("c (n r) w -> n c r w", r=RC)
                        eng(k).dma_start(
                            orr[:, :, r0 : r0 + HR, :],
                            ot[pb : pb + NCH, :, r0 : r0 + HR, :],
                        )

    nc.finalize()
    return nc


_nc_cache = None


def _get_nc():
    global _nc_cache
    if _nc_cache is None:
        _nc_cache = build_nc()
    return _nc_cache


def _prep_x(x):
    """Reflect-pad and chunk x on host: [S,C,H,W] f32 ->
    [S,NCH,C,RP,WP] f16 with per-chunk halo rows/cols materialized."""
    xh = x.astype(np.float16)
    xp = np.pad(xh, ((0, 0), (0, 0), (1, 1), (1, 1)), mode="reflect")
    # windows: [S, C, NCH, RP, WP] where chunk n covers padded rows 8n..8n+9
    win = np.lib.stride_tricks.sliding_window_view(xp, RP, axis=2)
    win = win[:, :, ::RC, :, :]  # [S, C, NCH, WP, RP]
    win = np.transpose(win, (0, 2, 1, 4, 3))  # [S, NCH, C, RP, WP]
    return np.ascontiguousarray(win)


def _run(x, sigma, trace=False):
    xe = _prep_x(np.asarray(x))
    sigma = np.ascontiguousarray(sigma).astype(np.float16)
    nc = _get_nc()
    in_maps = [
        {"x": xe[S * i : S * (i + 1)], "sigma": sigma[S * i : S * (i + 1)]}
        for i in range(N_CORES)
    ]
    res = run_bass_kernel_spmd(nc, in_maps, list(range(N_CORES)), trace=trace)
    out = np.concatenate([res.results[i]["out"] for i in range(N_CORES)], axis=0)
    return out.astype(np.float32, copy=False), res


def kernel(x, sigma):
    out, _ = _run(x, sigma)
    return out
